# revision 19
# baseline (speedup 1.0000x reference)
"""Trainium2 Bass kernel for a dense transformer block (attention + MLP, 2 LNs).

Reference: out = LN(x + attn(x)); out = LN(out + mlp(out)); B=2, L=2048, D=1024,
16 heads x 64, causal, RoPE, erf-GELU MLP with hidden 4096.

Sharding (zero-communication): 8 cores = 2 batches x 4 token-residues.
Core (b, r) owns tokens p === r (mod 4) of batch b — 512 tokens. It computes
K/V projections for the FULL sequence of its batch (duplicated work, uniform
across cores), attention for its own query rows (block-causal structure is
identical across cores; the intra-block diagonal mask depends on r and is
passed as data), then MLP + both LayerNorms on its own tokens. The host
scatters per-core outputs back into the full (2, 2048, 1024) tensor.

All activations live in transposed (channel-on-partition) layout; RoPE's
channel-pair mixing is handled by host-side de-interleaving of Wq/Wk columns
plus an on-chip 32-partition-block swap done with SBUF->SBUF DMA. Softmax
denominators ride along the attention-value matmul as a 65th ones-column of V.
Attention processes even/odd head pairs together (their K=64 score matmuls
occupy disjoint PE row groups and run concurrently) and is software-pipelined
two k-blocks deep so the PE never waits on the Scalar engine's exp.
The attention output bounces through DRAM between the attention and MLP halves
so their SBUF pools can reuse the same space. All DRAM inputs are pre-arranged
on the host into the exact SBUF tile layouts so every DMA moves long
contiguous per-partition rows.
"""

import contextlib
import sys
import types

import numpy as np
import ml_dtypes

# ---- shim the antenv.axon_hooks registry (missing in this container) so
# trace=True profiling works when a driver requests it -----------------------
if "antenv.axon_hooks" not in sys.modules:
    _hook_mod = types.ModuleType("antenv.axon_hooks")
    _hook_state = {"h": None}
    _hook_mod.set_axon_ntff_profile_hook = lambda h: _hook_state.__setitem__("h", h)
    _hook_mod.get_axon_ntff_profile_hook = lambda: _hook_state["h"]
    sys.modules["antenv.axon_hooks"] = _hook_mod
    try:
        import antenv

        antenv.axon_hooks = _hook_mod
    except ImportError:
        pass
    try:
        from trn_agent_boot.trn_boot import _ntff_profile_via_ctypes

        _hook_state["h"] = _ntff_profile_via_ctypes("/opt/axon/libaxon_pjrt.so")
    except Exception:
        pass

import concourse.bass as bass  # noqa: E402
import concourse.mybir as mybir  # noqa: E402
import concourse.tile as tile  # noqa: E402
from concourse import bacc  # noqa: E402
from concourse.bass_utils import run_bass_kernel_spmd  # noqa: E402

# ---- problem constants ------------------------------------------------------
B = 2
L = 2048
DIM = 1024
HEAD = 16
HD = 64
HID = 4 * DIM  # 4096
EPS = 1e-5
P = 128
NQ = L // 4          # 512 own tokens per core
CB = DIM // P        # 8 channel blocks
EB = HID // P        # 32 hidden blocks
NKB = L // P         # 16 k-token blocks
SC = 1.0 / np.sqrt(HD)

F32 = mybir.dt.float32
MM = mybir.dt.bfloat16           # matmul compute dtype
NP_MM = ml_dtypes.bfloat16

_CACHE = {}


# ---- device program ---------------------------------------------------------
def _build_program():
    nc = bacc.Bacc("TRN2", target_bir_lowering=False, debug=False,
                   enable_asserts=True, num_devices=8)

    d_xbT = nc.dram_tensor("xbT", [P, CB, L], MM, kind="ExternalInput").ap()
    d_xqm = nc.dram_tensor("xqTmm", [P, CB, NQ], MM, kind="ExternalInput").ap()
    d_xqf = nc.dram_tensor("xqTf", [P, CB, NQ], F32, kind="ExternalInput").ap()
    d_wq = nc.dram_tensor("Wq", [CB, P, CB, P], MM, kind="ExternalInput").ap()
    d_wk = nc.dram_tensor("Wk", [CB, P, CB, P], MM, kind="ExternalInput").ap()
    d_wv = nc.dram_tensor("Wv", [2, P, CB, 512], MM, kind="ExternalInput").ap()
    d_w1 = nc.dram_tensor("W1", [EB, P, CB, P], MM, kind="ExternalInput").ap()
    d_w2 = nc.dram_tensor("W2", [P, EB, DIM], MM, kind="ExternalInput").ap()
    d_cosq = nc.dram_tensor("cosq", [P, NQ], F32, kind="ExternalInput").ap()
    d_sinq = nc.dram_tensor("sinq", [P, NQ], F32, kind="ExternalInput").ap()
    d_cosk = nc.dram_tensor("cosk", [P, L], MM, kind="ExternalInput").ap()
    d_sink = nc.dram_tensor("sink", [P, L], MM, kind="ExternalInput").ap()
    d_maskL = nc.dram_tensor("maskL", [P, P], MM, kind="ExternalInput").ap()
    d_maskR = nc.dram_tensor("maskR", [P, 4 * P], MM, kind="ExternalInput").ap()
    d_gam = nc.dram_tensor("gammaT", [P, CB], F32, kind="ExternalInput").ap()
    d_bet = nc.dram_tensor("betaT", [P, CB], F32, kind="ExternalInput").ap()
    d_out = nc.dram_tensor("outT", [DIM, NQ], F32, kind="ExternalOutput").ap()

    AF = mybir.ActivationFunctionType

    with tile.TileContext(nc) as tc, contextlib.ExitStack() as ctx:
        small = ctx.enter_context(tc.tile_pool(name="small", bufs=1))
        stat = ctx.enter_context(tc.tile_pool(name="stat", bufs=1))
        dram = ctx.enter_context(tc.tile_pool(name="dram", bufs=1, space="DRAM"))

        gam = small.tile([P, CB], F32)
        nc.sync.dma_start(gam, d_gam)
        bet = small.tile([P, CB], F32)
        nc.sync.dma_start(bet, d_bet)
        ones128 = small.tile([P, P], MM)
        nc.vector.memset(ones128, 1.0)
        epst = small.tile([1, 1], F32)
        nc.vector.memset(epst, EPS)
        maskL = small.tile([P, P], MM)
        nc.sync.dma_start(maskL, d_maskL)
        maskR = small.tile([P, 4 * P], MM)
        nc.sync.dma_start(maskR, d_maskR)

        def layernorm(src_f32, dst_f32, dst_mm, mmpool, pool, pspool):
            """dst = LN(src) along channels; channels on partitions, 8 blocks."""
            src_mm = mmpool.tile([P, CB, 512], MM, tag="ln_mm")
            for cb in range(CB):
                nc.scalar.copy(src_mm[:, cb, :], src_f32[:, cb, :])
            ps_sum = pspool.tile([P, 512], F32, tag="ln_sum")
            ps_sq = pspool.tile([P, 512], F32, tag="ln_sq")
            for cb in range(CB):
                nc.tensor.matmul(ps_sum, lhsT=ones128, rhs=src_mm[:, cb, :],
                                 start=(cb == 0), stop=(cb == CB - 1))
            for cb in range(CB):
                sq = pool.tile([P, 512], MM, tag="ln_sq_mm")
                nc.vector.tensor_mul(sq, src_mm[:, cb, :], src_mm[:, cb, :])
                nc.tensor.matmul(ps_sq, lhsT=ones128, rhs=sq,
                                 start=(cb == 0), stop=(cb == CB - 1))
            mu = stat.tile([1, 512], F32, tag="mu")
            nc.vector.tensor_scalar_mul(mu, ps_sum[0:1, :], 1.0 / DIM)
            e2 = stat.tile([1, 512], F32, tag="e2")
            nc.vector.tensor_scalar_mul(e2, ps_sq[0:1, :], 1.0 / DIM)
            var = stat.tile([1, 512], F32, tag="var")
            nc.vector.tensor_mul(var, mu, mu)
            nc.vector.tensor_sub(var, e2, var)
            rstd = stat.tile([1, 512], F32, tag="rstd")
            nc.scalar.activation(out=rstd, in_=var, func=AF.Sqrt,
                                 bias=epst[0:1, :], scale=1.0)
            nc.vector.reciprocal(rstd, rstd)
            nmu = stat.tile([1, 512], F32, tag="nmu")
            nc.vector.tensor_mul(nmu, mu, rstd)
            nc.vector.tensor_scalar_mul(nmu, nmu, -1.0)
            rstd_b = stat.tile([P, 512], F32, tag="rstd_b")
            nc.gpsimd.partition_broadcast(rstd_b, rstd)
            nmu_b = stat.tile([P, 512], F32, tag="nmu_b")
            nc.gpsimd.partition_broadcast(nmu_b, nmu)
            for cb in range(CB):
                t1 = pool.tile([P, 512], F32, tag="ln_t1")
                nc.vector.tensor_mul(t1, src_f32[:, cb, :], rstd_b)
                nc.vector.tensor_add(t1, t1, nmu_b)
                nc.vector.tensor_scalar(
                    out=dst_f32[:, cb, :], in0=t1,
                    scalar1=gam[:, cb:cb + 1], scalar2=bet[:, cb:cb + 1],
                    op0=mybir.AluOpType.mult, op1=mybir.AluOpType.add)
                if dst_mm is not None:
                    nc.scalar.copy(dst_mm[:, cb, :], dst_f32[:, cb, :])

        # ======================= scope 1: QKV + attention ====================
        lnmm = ctx.enter_context(tc.tile_pool(name="lnmm", bufs=1))
        h1pool = ctx.enter_context(tc.tile_pool(name="h1pool", bufs=1))
        h1T = h1pool.tile([P, CB, NQ], F32)
        with tc.tile_pool(name="qkv", bufs=1) as qkv:
            kT = qkv.tile([P, CB, L], MM)
            qT = qkv.tile([P, CB, NQ], MM)
            vaug = qkv.tile([P, NKB, HEAD * (HD + 1)], MM)
            va3 = vaug.rearrange("p t (h c) -> p t h c", c=HD + 1)
            nc.vector.memset(va3[:, :, :, HD:HD + 1], 1.0)

            # ---------------- phase A: QKV projections + RoPE ----------------
            with (
                tc.tile_pool(name="xin", bufs=1) as xin,
                tc.tile_pool(name="wstream", bufs=2) as wstream,
                tc.tile_pool(name="ropetmp", bufs=2) as ropetmp,
                tc.tile_pool(name="tabs", bufs=1) as tabs,
                tc.tile_pool(name="psA", bufs=3, space="PSUM") as psA,
            ):
                # q first: small DMAs so the PE can start quickly
                xqm = xin.tile([P, CB, NQ], MM)
                nc.sync.dma_start(xqm, d_xqm)
                xbT = xin.tile([P, CB, L], MM)
                for t in range(4):
                    nc.sync.dma_start(xbT[:, :, t * 512:(t + 1) * 512],
                                      d_xbT[:, :, t * 512:(t + 1) * 512])
                cosq = tabs.tile([P, NQ], F32)
                nc.sync.dma_start(cosq, d_cosq)
                sinq = tabs.tile([P, NQ], F32)
                nc.sync.dma_start(sinq, d_sinq)
                cosk = tabs.tile([P, L], MM)
                nc.sync.dma_start(cosk, d_cosk)
                sink = tabs.tile([P, L], MM)
                nc.sync.dma_start(sink, d_sink)

                def rope_evac(ps, cosS, sinS, out_slice, width):
                    raw = ropetmp.tile([P, 512], MM, tag="rope_raw")
                    nc.scalar.copy(raw[:, :width], ps)
                    nc.vector.tensor_mul(out_slice, ps, cosS)
                    swp = ropetmp.tile([P, 512], MM, tag="rope_swp")
                    for g in range(4):
                        s = (g ^ 1) * 32
                        nc.sync.dma_start(swp[g * 32:(g + 1) * 32, :width],
                                          raw[s:s + 32, :width])
                    tmp = ropetmp.tile([P, 512], MM, tag="rope_tmp")
                    nc.vector.tensor_mul(tmp[:, :width], swp[:, :width], sinS)
                    nc.vector.tensor_add(out_slice, out_slice, tmp[:, :width])

                for cb in range(CB):
                    wq_t = wstream.tile([P, CB, P], MM, tag="wq")
                    nc.sync.dma_start(wq_t, d_wq[cb])
                    ps_q = psA.tile([P, 512], F32, tag="psA")
                    for kb in range(CB):
                        nc.tensor.matmul(ps_q, lhsT=wq_t[:, kb, :],
                                         rhs=xqm[:, kb, :],
                                         start=(kb == 0), stop=(kb == CB - 1))
                    rope_evac(ps_q, cosq, sinq, qT[:, cb, :], NQ)

                for cb in range(CB):
                    wk_t = wstream.tile([P, CB, P], MM, tag="wk")
                    nc.sync.dma_start(wk_t, d_wk[cb])
                    for t in range(L // 512):
                        ps_k = psA.tile([P, 512], F32, tag="psA")
                        for kb in range(CB):
                            nc.tensor.matmul(ps_k, lhsT=wk_t[:, kb, :],
                                             rhs=xbT[:, kb, t * 512:(t + 1) * 512],
                                             start=(kb == 0), stop=(kb == CB - 1))
                        rope_evac(ps_k, cosk[:, t * 512:(t + 1) * 512],
                                  sink[:, t * 512:(t + 1) * 512],
                                  kT[:, cb, t * 512:(t + 1) * 512], 512)

                for nch in range(2):
                    wv_t = wstream.tile([P, CB, 512], MM, tag="wv")
                    nc.sync.dma_start(wv_t, d_wv[nch])
                    for tb in range(NKB):
                        ps_v = psA.tile([P, 512], F32, tag="psA")
                        for kb in range(CB):
                            nc.tensor.matmul(ps_v, lhsT=xbT[:, kb, tb * P:(tb + 1) * P],
                                             rhs=wv_t[:, kb, :],
                                             start=(kb == 0), stop=(kb == CB - 1))
                        nc.scalar.copy(
                            va3[:, tb, nch * 8:(nch + 1) * 8, 0:HD],
                            ps_v.rearrange("p (h c) -> p h c", c=HD))

            # ---------------- phase B: attention (head pairs, 2-deep SW pipe)
            with (
                tc.tile_pool(name="attn", bufs=4) as attn,
                tc.tile_pool(name="xq2", bufs=1) as xq2,
                tc.tile_pool(name="psS", bufs=3, space="PSUM") as psS,
                tc.tile_pool(name="psO", bufs=1, space="PSUM") as psO,
            ):
                xqf = xq2.tile([P, CB, NQ], F32)
                nc.sync.dma_start(xqf, d_xqf)

                for hp in range(HEAD // 2):
                    hA, hB = 2 * hp, 2 * hp + 1
                    ps_oA = psO.tile([65, 512], F32, tag="ps_oA")
                    ps_oB = psO.tile([65, 512], F32, tag="ps_oB")
                    ps_s = [None] * NKB
                    ex = [None] * NKB

                    def scores(kb):
                        jmin = kb // 4
                        w = 512 - jmin * P
                        m = kb % 4
                        ps = psS.tile([P, 2, 512], F32, tag="ps_s")
                        nc.tensor.matmul(
                            ps[:, 0, :w],
                            lhsT=kT[0:64, hp, kb * P:(kb + 1) * P],
                            rhs=qT[0:64, hp, jmin * P:], start=True, stop=False)
                        nc.tensor.matmul(
                            ps[:, 1, :w],
                            lhsT=kT[64:128, hp, kb * P:(kb + 1) * P],
                            rhs=qT[64:128, hp, jmin * P:], start=True, stop=False)
                        nc.tensor.matmul(
                            ps[:, 0, 0:P], lhsT=maskL[0:64, :],
                            rhs=maskR[0:64, m * P:(m + 1) * P],
                            start=False, stop=True, skip_group_check=True)
                        nc.tensor.matmul(
                            ps[:, 1, 0:P], lhsT=maskL[64:128, :],
                            rhs=maskR[64:128, m * P:(m + 1) * P],
                            start=False, stop=True, skip_group_check=True)
                        ps_s[kb] = ps
                        e = attn.tile([P, 2, 512], MM, tag="ex")
                        nc.scalar.activation(out=e[:, :, :w], in_=ps[:, :, :w],
                                             func=AF.Exp, scale=float(SC))
                        ex[kb] = e

                    def av(kb):
                        jmin = kb // 4
                        w = 512 - jmin * P
                        nc.tensor.matmul(ps_oA[:, jmin * P:],
                                         lhsT=va3[:, kb, hA, :], rhs=ex[kb][:, 0, :w],
                                         start=(kb == 0), stop=(kb == NKB - 1))
                        nc.tensor.matmul(ps_oB[:, jmin * P:],
                                         lhsT=va3[:, kb, hB, :], rhs=ex[kb][:, 1, :w],
                                         start=(kb == 0), stop=(kb == NKB - 1))

                    scores(0)
                    scores(1)
                    for kb in range(NKB):
                        if kb + 2 < NKB:
                            scores(kb + 2)
                        av(kb)

                    for hx, ps_o in ((hA, ps_oA), (hB, ps_oB)):
                        po = (hx % 2) * 64
                        rec = attn.tile([1, 512], F32, tag="rec")
                        nc.vector.reciprocal(rec, ps_o[64:65, :])
                        rb = attn.tile([64, 512], F32, tag="rb")
                        nc.gpsimd.partition_broadcast(rb, rec)
                        nc.vector.tensor_mul(h1T[po:po + 64, hp, :],
                                             ps_o[0:64, :], rb)
                    nc.vector.tensor_add(h1T[:, hp, :], h1T[:, hp, :],
                                         xqf[:, hp, :])

        # ======================= scope 2: LN1 + MLP + LN2 ====================
        with (
            tc.tile_pool(name="w2res", bufs=1) as w2res,
            tc.tile_pool(name="hpool", bufs=1) as hpool,
            tc.tile_pool(name="lntmp", bufs=3) as lntmp,
            tc.tile_pool(name="psC", bufs=2, space="PSUM") as psC,
        ):
            w2 = w2res.tile([P, EB, DIM], MM)
            nc.sync.dma_start(w2, d_w2)

            h1nT = hpool.tile([P, CB, NQ], F32)
            h1nm = hpool.tile([P, CB, NQ], MM)

            layernorm(h1T, h1nT, h1nm, lnmm, lntmp, psC)

            # ---------------- phase D: MLP -----------------------------------
            with (
                tc.tile_pool(name="mlp", bufs=1) as mlp,
                tc.tile_pool(name="w1stream", bufs=3) as w1s,
                tc.tile_pool(name="psD", bufs=2, space="PSUM") as psD,
            ):
                aT = mlp.tile([P, EB, NQ], MM)
                for eb in range(EB):
                    w1_t = w1s.tile([P, CB, P], MM, tag="w1")
                    nc.sync.dma_start(w1_t, d_w1[eb])
                    ps_a = psD.tile([P, 512], F32, tag="ps_a")
                    for kb in range(CB):
                        nc.tensor.matmul(ps_a, lhsT=w1_t[:, kb, :],
                                         rhs=h1nm[:, kb, :],
                                         start=(kb == 0), stop=(kb == CB - 1))
                    nc.scalar.activation(out=aT[:, eb, :], in_=ps_a, func=AF.Gelu)

                h2T = mlp.tile([P, CB, NQ], F32)
                for cb in range(CB):
                    ps_2 = psD.tile([P, 512], F32, tag="ps_2")
                    for eb in range(EB):
                        nc.tensor.matmul(ps_2, lhsT=w2[:, eb, cb * P:(cb + 1) * P],
                                         rhs=aT[:, eb, :],
                                         start=(eb == 0), stop=(eb == EB - 1))
                    nc.vector.tensor_add(h2T[:, cb, :], ps_2, h1nT[:, cb, :])

                # -------- phase E: LN2 + output (in place on h2T) ------------
                layernorm(h2T, h2T, None, lnmm, lntmp, psC)
                for cb in range(CB):
                    nc.sync.dma_start(d_out[cb * P:(cb + 1) * P, :], h2T[:, cb, :])

    nc.compile()
    return nc


# ---- host-side preparation --------------------------------------------------
def _rope_tables():
    inv_freq = 1.0 / (10000.0 ** (np.arange(0, HD, 2, dtype=np.float32) / HD))
    pos = np.arange(L, dtype=np.float32)
    ang = np.einsum("i,j->ij", pos, inv_freq)  # (L, 32)
    return np.cos(ang).astype(np.float32), np.sin(ang).astype(np.float32)


def _prep_in_maps(x, Wq, Wk, Wv, W1, W2, gamma, beta):
    perm = np.concatenate(
        [h * HD + np.concatenate([np.arange(0, HD, 2), np.arange(1, HD, 2)])
         for h in range(HEAD)])
    Wq_p = Wq[:, perm]
    Wk_p = Wk[:, perm]
    cos, sin = _rope_tables()  # (L, 32)

    iidx = np.arange(P) % 32                  # table column per partition row
    sgn = np.where((np.arange(P) // 32) % 2 == 0, -1.0, 1.0).astype(np.float32)

    cosk = cos[:, iidx].T.astype(np.float32)              # (128, L)
    sink = (sin[:, iidx] * sgn[None, :]).T.astype(np.float32)

    gammaT = gamma.reshape(CB, P).T.astype(np.float32)    # [p, cb]
    betaT = beta.reshape(CB, P).T.astype(np.float32)

    def wlay(w, mblk):  # (DIM_in, M) -> (M//mblk, P, KB, mblk) contiguous
        kin = w.shape[0] // P
        return np.ascontiguousarray(
            w.reshape(kin, P, w.shape[1] // mblk, mblk).transpose(2, 1, 0, 3)
        ).astype(NP_MM)

    com = {
        "Wq": wlay(Wq_p, P), "Wk": wlay(Wk_p, P), "Wv": wlay(Wv, 512),
        "W1": wlay(W1, P),
        "W2": np.ascontiguousarray(
            W2.reshape(EB, P, DIM).transpose(1, 0, 2)).astype(NP_MM),
        "cosk": np.ascontiguousarray(cosk).astype(NP_MM),
        "sink": np.ascontiguousarray(sink).astype(NP_MM),
        "gammaT": np.ascontiguousarray(gammaT),
        "betaT": np.ascontiguousarray(betaT),

    }

    def xlay(xt, dt):  # (L', D) -> (P, CB, L') contiguous
        return np.ascontiguousarray(
            xt.T.reshape(CB, P, xt.shape[0]).transpose(1, 0, 2)).astype(dt)

    in_maps = []
    for core in range(8):
        b, rr = core // 4, core % 4
        pos_own = rr + 4 * np.arange(NQ)
        xb = x[b]                                     # (L, D)
        xq = xb[pos_own]                              # (NQ, D)
        cosq = cos[pos_own][:, iidx].T.astype(np.float32)          # (128, NQ)
        sinq = (sin[pos_own][:, iidx] * sgn[None, :]).T.astype(np.float32)
        # causal mask as additive rank-33 factorization: masked iff
        # t < tau0[u] + 32*m with tau0 = clip(ceil((u-rr)/4), 0, 32)
        u = np.arange(P)
        tau0 = np.clip(np.ceil((u - rr) / 4.0).astype(int), 0, 32)
        Lm = np.zeros((P, P), np.float32)           # lhsT: [k, u]
        Lm[tau0, np.arange(P)] = 1.0                # rows 0..32
        Lm[64 + tau0, np.arange(P)] = 0.0           # (filled below)
        maskL = np.zeros((P, P), np.float32)
        maskL[tau0, np.arange(P)] = 1.0
        maskL[64 + tau0, np.arange(P)] = 1.0
        jj = np.arange(64)[:, None]
        tt = np.arange(P)[None, :]
        maskR = np.zeros((P, 4 * P), np.float32)
        for m_ in range(4):
            blk = np.where((tt < jj + 32 * m_) & (jj <= 32), -8000.0, 0.0)
            maskR[0:64, m_ * P:(m_ + 1) * P] = blk
            maskR[64:128, m_ * P:(m_ + 1) * P] = blk
        m = dict(com)
        m["xbT"] = xlay(xb, NP_MM)
        m["xqTmm"] = xlay(xq, NP_MM)
        m["xqTf"] = xlay(xq, np.float32)
        m["cosq"] = np.ascontiguousarray(cosq)
        m["sinq"] = np.ascontiguousarray(sinq)
        m["maskL"] = np.ascontiguousarray(maskL).astype(NP_MM)
        m["maskR"] = np.ascontiguousarray(maskR).astype(NP_MM)
        in_maps.append(m)
    return in_maps


def _assemble(results):
    out = np.empty((B, L, DIM), dtype=np.float32)
    for core in range(8):
        b, rr = core // 4, core % 4
        out[b, rr::4, :] = results[core]["outT"].T
    return out


def _get_program():
    if "nc" not in _CACHE:
        _CACHE["nc"] = _build_program()
    return _CACHE["nc"]


def run(in_maps, trace=False, **kw):
    nc = _get_program()
    return run_bass_kernel_spmd(nc, in_maps, core_ids=list(range(8)),
                                trace=trace, **kw)


def kernel(x, Wq, bq, Wk, bk, Wv, bv, W1, b1, W2, b2, gamma, beta):
    for name, b_ in (("bq", bq), ("bk", bk), ("bv", bv), ("b1", b1), ("b2", b2)):
        if np.abs(np.asarray(b_)).max() != 0.0:
            raise NotImplementedError(f"nonzero bias {name} not supported")
    x = np.asarray(x, dtype=np.float32)
    in_maps = _prep_in_maps(
        x, np.asarray(Wq), np.asarray(Wk), np.asarray(Wv),
        np.asarray(W1), np.asarray(W2), np.asarray(gamma), np.asarray(beta))
    res = run(in_maps, trace=False)
    return _assemble(res.results)


# revision 20
# speedup vs baseline: 1.1113x; 1.1113x over previous
"""Trainium2 Bass kernel for a dense transformer block (attention + MLP, 2 LNs).

Reference: out = LN(x + attn(x)); out = LN(out + mlp(out)); B=2, L=2048, D=1024,
16 heads x 64, causal, RoPE, erf-GELU MLP with hidden 4096.

Sharding (zero-communication): 8 cores = 2 batches x 4 token-residues.
Core (b, r) owns tokens p === r (mod 4) of batch b — 512 tokens. It computes
K/V projections for the FULL sequence of its batch (duplicated work, uniform
across cores), attention for its own query rows (block-causal structure is
identical across cores; the intra-block diagonal mask depends on r and is
passed as data), then MLP + both LayerNorms on its own tokens. The host
scatters per-core outputs back into the full (2, 2048, 1024) tensor.

All activations live in transposed (channel-on-partition) layout; RoPE's
channel-pair mixing is handled by host-side de-interleaving of Wq/Wk columns
plus an on-chip 32-partition-block swap done with SBUF->SBUF DMA. Softmax
denominators ride along the attention-value matmul as a 65th ones-column of V.
Attention processes even/odd head pairs together (their K=64 score matmuls
occupy disjoint PE row groups and run concurrently) and is software-pipelined
two k-blocks deep so the PE never waits on the Scalar engine's exp.
The attention output bounces through DRAM between the attention and MLP halves
so their SBUF pools can reuse the same space. All DRAM inputs are pre-arranged
on the host into the exact SBUF tile layouts so every DMA moves long
contiguous per-partition rows.
"""

import contextlib
import sys
import types

import numpy as np
import ml_dtypes

# ---- shim the antenv.axon_hooks registry (missing in this container) so
# trace=True profiling works when a driver requests it -----------------------
if "antenv.axon_hooks" not in sys.modules:
    _hook_mod = types.ModuleType("antenv.axon_hooks")
    _hook_state = {"h": None}
    _hook_mod.set_axon_ntff_profile_hook = lambda h: _hook_state.__setitem__("h", h)
    _hook_mod.get_axon_ntff_profile_hook = lambda: _hook_state["h"]
    sys.modules["antenv.axon_hooks"] = _hook_mod
    try:
        import antenv

        antenv.axon_hooks = _hook_mod
    except ImportError:
        pass
    try:
        from trn_agent_boot.trn_boot import _ntff_profile_via_ctypes

        _hook_state["h"] = _ntff_profile_via_ctypes("/opt/axon/libaxon_pjrt.so")
    except Exception:
        pass

import concourse.bass as bass  # noqa: E402
import concourse.mybir as mybir  # noqa: E402
import concourse.tile as tile  # noqa: E402
from concourse import bacc  # noqa: E402
from concourse.bass_utils import run_bass_kernel_spmd  # noqa: E402

# ---- problem constants ------------------------------------------------------
B = 2
L = 2048
DIM = 1024
HEAD = 16
HD = 64
HID = 4 * DIM  # 4096
EPS = 1e-5
P = 128
NQ = L // 4          # 512 own tokens per core
CB = DIM // P        # 8 channel blocks
EB = HID // P        # 32 hidden blocks
NKB = L // P         # 16 k-token blocks
SC = 1.0 / np.sqrt(HD)

F32 = mybir.dt.float32
MM = mybir.dt.bfloat16           # matmul compute dtype
NP_MM = ml_dtypes.bfloat16

_CACHE = {}


# ---- device program ---------------------------------------------------------
def _build_program():
    nc = bacc.Bacc("TRN2", target_bir_lowering=False, debug=False,
                   enable_asserts=True, num_devices=8)

    d_xbT = nc.dram_tensor("xbT", [P, CB, L], MM, kind="ExternalInput").ap()
    d_xqm = nc.dram_tensor("xqTmm", [P, CB, NQ], MM, kind="ExternalInput").ap()
    d_xqf = nc.dram_tensor("xqTf", [P, CB, NQ], F32, kind="ExternalInput").ap()
    d_wq = nc.dram_tensor("Wq", [CB, P, CB, P], MM, kind="ExternalInput").ap()
    d_wk = nc.dram_tensor("Wk", [CB, P, CB, P], MM, kind="ExternalInput").ap()
    d_wv = nc.dram_tensor("Wv", [2, P, CB, 512], MM, kind="ExternalInput").ap()
    d_w1 = nc.dram_tensor("W1", [EB, P, CB, P], MM, kind="ExternalInput").ap()
    d_w2 = nc.dram_tensor("W2", [P, EB, DIM], MM, kind="ExternalInput").ap()
    d_cosq = nc.dram_tensor("cosq", [P, NQ], F32, kind="ExternalInput").ap()
    d_sinq = nc.dram_tensor("sinq", [P, NQ], F32, kind="ExternalInput").ap()
    d_cosk = nc.dram_tensor("cosk", [P, L], MM, kind="ExternalInput").ap()
    d_sink = nc.dram_tensor("sink", [P, L], MM, kind="ExternalInput").ap()
    d_maskL = nc.dram_tensor("maskL", [P, P], MM, kind="ExternalInput").ap()
    d_maskR = nc.dram_tensor("maskR", [P, 4 * P], MM, kind="ExternalInput").ap()
    d_gam = nc.dram_tensor("gammaT", [P, CB], F32, kind="ExternalInput").ap()
    d_bet = nc.dram_tensor("betaT", [P, CB], F32, kind="ExternalInput").ap()
    d_out = nc.dram_tensor("outT", [DIM, NQ], F32, kind="ExternalOutput").ap()

    AF = mybir.ActivationFunctionType

    with tile.TileContext(nc) as tc, contextlib.ExitStack() as ctx:
        small = ctx.enter_context(tc.tile_pool(name="small", bufs=1))
        stat = ctx.enter_context(tc.tile_pool(name="stat", bufs=1))
        dram = ctx.enter_context(tc.tile_pool(name="dram", bufs=1, space="DRAM"))

        gam = small.tile([P, CB], F32)
        nc.sync.dma_start(gam, d_gam)
        bet = small.tile([P, CB], F32)
        nc.sync.dma_start(bet, d_bet)
        ones128 = small.tile([P, P], MM)
        nc.vector.memset(ones128, 1.0)
        epst = small.tile([1, 1], F32)
        nc.vector.memset(epst, EPS)
        maskL = small.tile([P, P], MM)
        nc.sync.dma_start(maskL, d_maskL)
        maskR = small.tile([P, 4 * P], MM)
        nc.sync.dma_start(maskR, d_maskR)

        def layernorm(src_f32, dst_f32, dst_mm, mmpool, pool, pspool):
            """dst = LN(src) along channels; channels on partitions, 8 blocks."""
            src_mm = mmpool.tile([P, CB, 512], MM, tag="ln_mm")
            for cb in range(CB):
                nc.scalar.copy(src_mm[:, cb, :], src_f32[:, cb, :])
            ps_sum = pspool.tile([P, 512], F32, tag="ln_sum")
            ps_sq = pspool.tile([P, 512], F32, tag="ln_sq")
            for cb in range(CB):
                nc.tensor.matmul(ps_sum, lhsT=ones128, rhs=src_mm[:, cb, :],
                                 start=(cb == 0), stop=(cb == CB - 1))
            for cb in range(CB):
                sq = pool.tile([P, 512], MM, tag="ln_sq_mm")
                nc.vector.tensor_mul(sq, src_mm[:, cb, :], src_mm[:, cb, :])
                nc.tensor.matmul(ps_sq, lhsT=ones128, rhs=sq,
                                 start=(cb == 0), stop=(cb == CB - 1))
            mu = stat.tile([1, 512], F32, tag="mu")
            nc.vector.tensor_scalar_mul(mu, ps_sum[0:1, :], 1.0 / DIM)
            e2 = stat.tile([1, 512], F32, tag="e2")
            nc.vector.tensor_scalar_mul(e2, ps_sq[0:1, :], 1.0 / DIM)
            var = stat.tile([1, 512], F32, tag="var")
            nc.vector.tensor_mul(var, mu, mu)
            nc.vector.tensor_sub(var, e2, var)
            rstd = stat.tile([1, 512], F32, tag="rstd")
            nc.scalar.activation(out=rstd, in_=var, func=AF.Sqrt,
                                 bias=epst[0:1, :], scale=1.0)
            nc.vector.reciprocal(rstd, rstd)
            nmu = stat.tile([1, 512], F32, tag="nmu")
            nc.vector.tensor_mul(nmu, mu, rstd)
            nc.vector.tensor_scalar_mul(nmu, nmu, -1.0)
            rstd_b = stat.tile([P, 512], F32, tag="rstd_b")
            nc.gpsimd.partition_broadcast(rstd_b, rstd)
            nmu_b = stat.tile([P, 512], F32, tag="nmu_b")
            nc.gpsimd.partition_broadcast(nmu_b, nmu)
            for cb in range(CB):
                t1 = pool.tile([P, 512], F32, tag="ln_t1")
                nc.vector.tensor_mul(t1, src_f32[:, cb, :], rstd_b)
                nc.vector.tensor_add(t1, t1, nmu_b)
                nc.vector.tensor_scalar(
                    out=dst_f32[:, cb, :], in0=t1,
                    scalar1=gam[:, cb:cb + 1], scalar2=bet[:, cb:cb + 1],
                    op0=mybir.AluOpType.mult, op1=mybir.AluOpType.add)
                if dst_mm is not None:
                    nc.scalar.copy(dst_mm[:, cb, :], dst_f32[:, cb, :])

        # ======================= scope 1: QKV + attention ====================
        lnmm = ctx.enter_context(tc.tile_pool(name="lnmm", bufs=1))
        h1pool = ctx.enter_context(tc.tile_pool(name="h1pool", bufs=1))
        h1T = h1pool.tile([P, CB, NQ], F32)
        with tc.tile_pool(name="qkv", bufs=1) as qkv:
            kT = qkv.tile([P, CB, L], MM)
            qT = qkv.tile([P, CB, NQ], MM)
            vaug = qkv.tile([P, NKB, HEAD * (HD + 1)], MM)
            va3 = vaug.rearrange("p t (h c) -> p t h c", c=HD + 1)
            nc.vector.memset(va3[:, :, :, HD:HD + 1], 1.0)

            # ---------------- phase A: QKV projections + RoPE ----------------
            with (
                tc.tile_pool(name="xin", bufs=1) as xin,
                tc.tile_pool(name="wstream", bufs=2) as wstream,
                tc.tile_pool(name="ropetmp", bufs=2) as ropetmp,
                tc.tile_pool(name="tabs", bufs=1) as tabs,
                tc.tile_pool(name="psA", bufs=3, space="PSUM") as psA,
            ):
                # q first: small DMAs so the PE can start quickly
                xqm = xin.tile([P, CB, NQ], MM)
                nc.sync.dma_start(xqm, d_xqm)
                xbT = xin.tile([P, CB, L], MM)
                for t in range(4):
                    nc.sync.dma_start(xbT[:, :, t * 512:(t + 1) * 512],
                                      d_xbT[:, :, t * 512:(t + 1) * 512])
                cosq = tabs.tile([P, NQ], F32)
                nc.sync.dma_start(cosq, d_cosq)
                sinq = tabs.tile([P, NQ], F32)
                nc.sync.dma_start(sinq, d_sinq)
                cosk = tabs.tile([P, L], MM)
                nc.sync.dma_start(cosk, d_cosk)
                sink = tabs.tile([P, L], MM)
                nc.sync.dma_start(sink, d_sink)

                def rope_evac(ps, cosS, sinS, out_slice, width):
                    raw = ropetmp.tile([P, 512], MM, tag="rope_raw")
                    nc.scalar.copy(raw[:, :width], ps)
                    nc.vector.tensor_mul(out_slice, ps, cosS)
                    swp = ropetmp.tile([P, 512], MM, tag="rope_swp")
                    for g in range(4):
                        s = (g ^ 1) * 32
                        nc.sync.dma_start(swp[g * 32:(g + 1) * 32, :width],
                                          raw[s:s + 32, :width])
                    tmp = ropetmp.tile([P, 512], MM, tag="rope_tmp")
                    nc.vector.tensor_mul(tmp[:, :width], swp[:, :width], sinS)
                    nc.vector.tensor_add(out_slice, out_slice, tmp[:, :width])

                for cb in range(CB):
                    wq_t = wstream.tile([P, CB, P], MM, tag="wq")
                    nc.sync.dma_start(wq_t, d_wq[cb])
                    ps_q = psA.tile([P, 512], F32, tag="psA")
                    for kb in range(CB):
                        nc.tensor.matmul(ps_q, lhsT=wq_t[:, kb, :],
                                         rhs=xqm[:, kb, :],
                                         start=(kb == 0), stop=(kb == CB - 1))
                    rope_evac(ps_q, cosq, sinq, qT[:, cb, :], NQ)

                for cb in range(CB):
                    wk_t = wstream.tile([P, CB, P], MM, tag="wk")
                    nc.sync.dma_start(wk_t, d_wk[cb])
                    for t in range(L // 512):
                        ps_k = psA.tile([P, 512], F32, tag="psA")
                        for kb in range(CB):
                            nc.tensor.matmul(ps_k, lhsT=wk_t[:, kb, :],
                                             rhs=xbT[:, kb, t * 512:(t + 1) * 512],
                                             start=(kb == 0), stop=(kb == CB - 1))
                        rope_evac(ps_k, cosk[:, t * 512:(t + 1) * 512],
                                  sink[:, t * 512:(t + 1) * 512],
                                  kT[:, cb, t * 512:(t + 1) * 512], 512)

                for nch in range(2):
                    wv_t = wstream.tile([P, CB, 512], MM, tag="wv")
                    nc.sync.dma_start(wv_t, d_wv[nch])
                    for tb in range(NKB):
                        ps_v = psA.tile([P, 512], F32, tag="psA")
                        for kb in range(CB):
                            nc.tensor.matmul(ps_v, lhsT=xbT[:, kb, tb * P:(tb + 1) * P],
                                             rhs=wv_t[:, kb, :],
                                             start=(kb == 0), stop=(kb == CB - 1))
                        nc.scalar.copy(
                            va3[:, tb, nch * 8:(nch + 1) * 8, 0:HD],
                            ps_v.rearrange("p (h c) -> p h c", c=HD))

            # ---------------- phase B: attention (head pairs, 2-deep SW pipe)
            with (
                tc.tile_pool(name="attn", bufs=4) as attn,
                tc.tile_pool(name="xq2", bufs=1) as xq2,
                tc.tile_pool(name="psS", bufs=3, space="PSUM") as psS,
                tc.tile_pool(name="psO", bufs=1, space="PSUM") as psO,
            ):
                xqf = xq2.tile([P, CB, NQ], F32)
                nc.sync.dma_start(xqf, d_xqf)

                for hp in range(HEAD // 2):
                    hA, hB = 2 * hp, 2 * hp + 1
                    ps_oA = psO.tile([65, 512], F32, tag="ps_oA")
                    ps_oB = psO.tile([65, 512], F32, tag="ps_oB")
                    ps_s = [None] * NKB
                    ex = [None] * NKB

                    def scores(kb):
                        jmin = kb // 4
                        w = 512 - jmin * P
                        m = kb % 4
                        ps = psS.tile([P, 2, 512], F32, tag="ps_s")
                        nc.tensor.matmul(
                            ps[:, 0, :w],
                            lhsT=kT[0:64, hp, kb * P:(kb + 1) * P],
                            rhs=qT[0:64, hp, jmin * P:], start=True, stop=False)
                        nc.tensor.matmul(
                            ps[:, 1, :w],
                            lhsT=kT[64:128, hp, kb * P:(kb + 1) * P],
                            rhs=qT[64:128, hp, jmin * P:], start=True, stop=False)
                        nc.tensor.matmul(
                            ps[:, 0, 0:P], lhsT=maskL[0:64, :],
                            rhs=maskR[0:64, m * P:(m + 1) * P],
                            start=False, stop=True, skip_group_check=True)
                        nc.tensor.matmul(
                            ps[:, 1, 0:P], lhsT=maskL[64:128, :],
                            rhs=maskR[64:128, m * P:(m + 1) * P],
                            start=False, stop=True, skip_group_check=True)
                        ps_s[kb] = ps
                        e = attn.tile([P, 2, 512], MM, tag="ex")
                        nc.scalar.activation(out=e[:, :, :w], in_=ps[:, :, :w],
                                             func=AF.Exp, scale=float(SC))
                        ex[kb] = e

                    def av(kb):
                        jmin = kb // 4
                        w = 512 - jmin * P
                        nc.tensor.matmul(ps_oA[:, jmin * P:],
                                         lhsT=va3[:, kb, hA, :], rhs=ex[kb][:, 0, :w],
                                         start=(kb == 0), stop=(kb == NKB - 1))
                        nc.tensor.matmul(ps_oB[:, jmin * P:],
                                         lhsT=va3[:, kb, hB, :], rhs=ex[kb][:, 1, :w],
                                         start=(kb == 0), stop=(kb == NKB - 1))

                    scores(0)
                    scores(1)
                    for kb in range(NKB):
                        if kb + 2 < NKB:
                            scores(kb + 2)
                        av(kb)

                    for hx, ps_o in ((hA, ps_oA), (hB, ps_oB)):
                        po = (hx % 2) * 64
                        cpy = attn.tile([65, 512], F32, tag="ocpy")
                        nc.vector.tensor_copy(cpy, ps_o)
                        rec = attn.tile([1, 512], F32, tag="rec")
                        nc.vector.reciprocal(rec, cpy[64:65, :])
                        rb = attn.tile([64, 512], F32, tag="rb")
                        nc.gpsimd.partition_broadcast(rb, rec)
                        nc.vector.tensor_mul(h1T[po:po + 64, hp, :],
                                             cpy[0:64, :], rb)
                    nc.vector.tensor_add(h1T[:, hp, :], h1T[:, hp, :],
                                         xqf[:, hp, :])

        # ======================= scope 2: LN1 + MLP + LN2 ====================
        with (
            tc.tile_pool(name="w2res", bufs=1) as w2res,
            tc.tile_pool(name="hpool", bufs=1) as hpool,
            tc.tile_pool(name="lntmp", bufs=3) as lntmp,
            tc.tile_pool(name="psC", bufs=2, space="PSUM") as psC,
        ):
            w2 = w2res.tile([P, EB, DIM], MM)
            nc.sync.dma_start(w2, d_w2)

            h1nT = hpool.tile([P, CB, NQ], F32)
            h1nm = hpool.tile([P, CB, NQ], MM)

            layernorm(h1T, h1nT, h1nm, lnmm, lntmp, psC)

            # ---------------- phase D: MLP -----------------------------------
            with (
                tc.tile_pool(name="mlp", bufs=1) as mlp,
                tc.tile_pool(name="w1stream", bufs=3) as w1s,
                tc.tile_pool(name="psD", bufs=2, space="PSUM") as psD,
            ):
                aT = mlp.tile([P, EB, NQ], MM)
                for eb in range(EB):
                    w1_t = w1s.tile([P, CB, P], MM, tag="w1")
                    nc.sync.dma_start(w1_t, d_w1[eb])
                    ps_a = psD.tile([P, 512], F32, tag="ps_a")
                    for kb in range(CB):
                        nc.tensor.matmul(ps_a, lhsT=w1_t[:, kb, :],
                                         rhs=h1nm[:, kb, :],
                                         start=(kb == 0), stop=(kb == CB - 1))
                    nc.scalar.activation(out=aT[:, eb, :], in_=ps_a, func=AF.Gelu)

                h2T = mlp.tile([P, CB, NQ], F32)
                for cb in range(CB):
                    ps_2 = psD.tile([P, 512], F32, tag="ps_2")
                    for eb in range(EB):
                        nc.tensor.matmul(ps_2, lhsT=w2[:, eb, cb * P:(cb + 1) * P],
                                         rhs=aT[:, eb, :],
                                         start=(eb == 0), stop=(eb == EB - 1))
                    nc.vector.tensor_add(h2T[:, cb, :], ps_2, h1nT[:, cb, :])

                # -------- phase E: LN2 + output (in place on h2T) ------------
                layernorm(h2T, h2T, None, lnmm, lntmp, psC)
                for cb in range(CB):
                    nc.sync.dma_start(d_out[cb * P:(cb + 1) * P, :], h2T[:, cb, :])

    nc.compile()
    return nc


# ---- host-side preparation --------------------------------------------------
def _rope_tables():
    inv_freq = 1.0 / (10000.0 ** (np.arange(0, HD, 2, dtype=np.float32) / HD))
    pos = np.arange(L, dtype=np.float32)
    ang = np.einsum("i,j->ij", pos, inv_freq)  # (L, 32)
    return np.cos(ang).astype(np.float32), np.sin(ang).astype(np.float32)


def _prep_in_maps(x, Wq, Wk, Wv, W1, W2, gamma, beta):
    perm = np.concatenate(
        [h * HD + np.concatenate([np.arange(0, HD, 2), np.arange(1, HD, 2)])
         for h in range(HEAD)])
    Wq_p = Wq[:, perm]
    Wk_p = Wk[:, perm]
    cos, sin = _rope_tables()  # (L, 32)

    iidx = np.arange(P) % 32                  # table column per partition row
    sgn = np.where((np.arange(P) // 32) % 2 == 0, -1.0, 1.0).astype(np.float32)

    cosk = cos[:, iidx].T.astype(np.float32)              # (128, L)
    sink = (sin[:, iidx] * sgn[None, :]).T.astype(np.float32)

    gammaT = gamma.reshape(CB, P).T.astype(np.float32)    # [p, cb]
    betaT = beta.reshape(CB, P).T.astype(np.float32)

    def wlay(w, mblk):  # (DIM_in, M) -> (M//mblk, P, KB, mblk) contiguous
        kin = w.shape[0] // P
        return np.ascontiguousarray(
            w.reshape(kin, P, w.shape[1] // mblk, mblk).transpose(2, 1, 0, 3)
        ).astype(NP_MM)

    com = {
        "Wq": wlay(Wq_p, P), "Wk": wlay(Wk_p, P), "Wv": wlay(Wv, 512),
        "W1": wlay(W1, P),
        "W2": np.ascontiguousarray(
            W2.reshape(EB, P, DIM).transpose(1, 0, 2)).astype(NP_MM),
        "cosk": np.ascontiguousarray(cosk).astype(NP_MM),
        "sink": np.ascontiguousarray(sink).astype(NP_MM),
        "gammaT": np.ascontiguousarray(gammaT),
        "betaT": np.ascontiguousarray(betaT),

    }

    def xlay(xt, dt):  # (L', D) -> (P, CB, L') contiguous
        return np.ascontiguousarray(
            xt.T.reshape(CB, P, xt.shape[0]).transpose(1, 0, 2)).astype(dt)

    in_maps = []
    for core in range(8):
        b, rr = core // 4, core % 4
        pos_own = rr + 4 * np.arange(NQ)
        xb = x[b]                                     # (L, D)
        xq = xb[pos_own]                              # (NQ, D)
        cosq = cos[pos_own][:, iidx].T.astype(np.float32)          # (128, NQ)
        sinq = (sin[pos_own][:, iidx] * sgn[None, :]).T.astype(np.float32)
        # causal mask as additive rank-33 factorization: masked iff
        # t < tau0[u] + 32*m with tau0 = clip(ceil((u-rr)/4), 0, 32)
        u = np.arange(P)
        tau0 = np.clip(np.ceil((u - rr) / 4.0).astype(int), 0, 32)
        Lm = np.zeros((P, P), np.float32)           # lhsT: [k, u]
        Lm[tau0, np.arange(P)] = 1.0                # rows 0..32
        Lm[64 + tau0, np.arange(P)] = 0.0           # (filled below)
        maskL = np.zeros((P, P), np.float32)
        maskL[tau0, np.arange(P)] = 1.0
        maskL[64 + tau0, np.arange(P)] = 1.0
        jj = np.arange(64)[:, None]
        tt = np.arange(P)[None, :]
        maskR = np.zeros((P, 4 * P), np.float32)
        for m_ in range(4):
            blk = np.where((tt < jj + 32 * m_) & (jj <= 32), -8000.0, 0.0)
            maskR[0:64, m_ * P:(m_ + 1) * P] = blk
            maskR[64:128, m_ * P:(m_ + 1) * P] = blk
        m = dict(com)
        m["xbT"] = xlay(xb, NP_MM)
        m["xqTmm"] = xlay(xq, NP_MM)
        m["xqTf"] = xlay(xq, np.float32)
        m["cosq"] = np.ascontiguousarray(cosq)
        m["sinq"] = np.ascontiguousarray(sinq)
        m["maskL"] = np.ascontiguousarray(maskL).astype(NP_MM)
        m["maskR"] = np.ascontiguousarray(maskR).astype(NP_MM)
        in_maps.append(m)
    return in_maps


def _assemble(results):
    out = np.empty((B, L, DIM), dtype=np.float32)
    for core in range(8):
        b, rr = core // 4, core % 4
        out[b, rr::4, :] = results[core]["outT"].T
    return out


def _get_program():
    if "nc" not in _CACHE:
        _CACHE["nc"] = _build_program()
    return _CACHE["nc"]


def run(in_maps, trace=False, **kw):
    nc = _get_program()
    return run_bass_kernel_spmd(nc, in_maps, core_ids=list(range(8)),
                                trace=trace, **kw)


def kernel(x, Wq, bq, Wk, bk, Wv, bv, W1, b1, W2, b2, gamma, beta):
    for name, b_ in (("bq", bq), ("bk", bk), ("bv", bv), ("b1", b1), ("b2", b2)):
        if np.abs(np.asarray(b_)).max() != 0.0:
            raise NotImplementedError(f"nonzero bias {name} not supported")
    x = np.asarray(x, dtype=np.float32)
    in_maps = _prep_in_maps(
        x, np.asarray(Wq), np.asarray(Wk), np.asarray(Wv),
        np.asarray(W1), np.asarray(W2), np.asarray(gamma), np.asarray(beta))
    res = run(in_maps, trace=False)
    return _assemble(res.results)
